# revision 4
# baseline (speedup 1.0000x reference)
"""Trainium2 Bass kernel v2 for nn_BiGNN (gnn_message_passing).

Math: p_i = max_k relu(bn_i(feat_i[idx_i] @ Wg_i)); out = relu(bn_o(cat @ Wout)).
BN is folded on the host (sign into Wg columns, |scale| into head weights).

Strategy (8 cores, data-parallel over the 50k target voxels, NT=6272
padded targets per core).  The neighbor gather runs on the HOST: the
device receives a per-core, bf16, channel-major expanded table
E[(s2 ch 0..63 | s1 ch 0..31), col] in exact consumption order.

  * ONE streaming pass: stationary W_pad [128,128] holds BOTH scale MLPs
    block-diagonally (rows 0:96 = E channels, rows 96:128 = zero pad;
    out ch 0:64 = z2, 64:128 = z1), so each gathered (t,k) column is
    streamed through the PE once (v1 streamed each column twice).
    Contraction is padded to 128 rows because 96-row stationaries lose
    FWL and run ~2x slower (426 vs 216 ns / 512-col matmul, measured);
    the 32 pad rows of each E tile are zeroed once per buffer by DMA.
  * PSUM drain is the structural wall (PE may only write PSUM; DVE reads
    PSUM at 1 elem/lane/cyc @0.96GHz, ACT at 1 elem/lane/cyc @1.2GHz;
    GPSIMD has no PSUM port and its elementwise ops do not lower on this
    toolchain).  Per 2048-col step one of two routes, ratio chosen so
    DVE and ACT busy times balance:
      D : DVE tensor_reduce(max) straight from PSUM + fused relu(m+b)
          via dual-op tensor_scalar (per-partition bias AP)
      Av: ACT activation(Relu, bias) drain to bf16 SBUF + DVE 2x-packed
          max tree; adjacent Av steps share one zc tile so the tree runs
          at double FD (fewer instruction overheads)
  * Head: per 512-target chunk, a 2-matmul f32 PSUM accumulation group
    (wCB^T rT + wA^T fL, both weights zero-padded to [128,128] to keep
    FWL) then relu(x+bo) drained via DVE tensor_scalar or ACT.
  * ~28 short warmup matmuls run while the first E columns load so the
    stream hits the PE warm (HAM K=8/8).

Host prep is pure data movement + weight folding; all FLOPs stay on
device.  The (t,k) -> E column order: step i covers targets
i*128 + m*32 + u (m = 512-col matmul index, u = 0..31), k innermost.
Output leaves as oT [64, NT] bf16; the host transposes and casts.
"""

import os
import sys
import numpy as np
import ml_dtypes

for _p in ("/opt/trn_rl_repo", "/opt/pypackages"):
    if os.path.isdir(_p) and _p not in sys.path:
        sys.path.append(_p)

import concourse.bass as bass
import concourse.mybir as mybir
import concourse.tile as tile
from concourse import bacc

EPS = 1e-3
N_CORES = 8
F32 = mybir.dt.float32
BF16 = mybir.dt.bfloat16
NPBF16 = ml_dtypes.bfloat16

# problem dims
N_LAST, M1, M2, K = 50000, 200000, 100000, 16
C1, C2, CL, CG = 32, 64, 64, 64

NT = 6272                  # padded targets per core (49 * 128)
STEP_T = 128               # targets per step
NSTEP = NT // STEP_T       # 49
STEP_C = STEP_T * K        # 2048 E columns per step
COLS = NT * K              # 100352
LOAD_STEPS = 4             # steps per E load
LOAD_C = STEP_C * LOAD_STEPS
ECH = C2 + C1              # 96 real channels

# drain route per step: "D" (DVE reduce) / "Av" (ACT relu-drain + DVE tree),
# tuned so DVE and ACT engine busy times balance (GPSIMD cannot run any
# multi-tensor op through this toolchain; measured 14.8us/instr on 1-input
# tensor_scalar, so it is useless for compute here).
ROUTES = []
_pat = ["Av", "Av", "D", "Av", "Av"]
for i in range(NSTEP):
    ROUTES.append(_pat[i % len(_pat)])
# adjacent Av steps are drained into a shared zc pair tile; the max tree
# then runs once per pair with double-size DVE instructions
TREE_PAIR = {}
_i = 0
while _i < NSTEP:
    if ROUTES[_i] == "Av" and _i + 1 < NSTEP and ROUTES[_i + 1] == "Av":
        TREE_PAIR[_i] = 0      # first of pair
        TREE_PAIR[_i + 1] = 1  # second of pair
        _i += 2
    else:
        _i += 1

# head drain engine per chunk: alternate DVE/ACT
HEAD_CHUNK = 512


def _head_chunks():
    out, c0 = [], 0
    while c0 < NT:
        w = min(HEAD_CHUNK, NT - c0)
        out.append((c0, w))
        c0 += w
    return out


def _emit(tc, io):
    nc = tc.nc

    with (
        tc.tile_pool(name="consts", bufs=1) as consts,
        tc.tile_pool(name="persist", bufs=1) as persist,
        tc.tile_pool(name="load", bufs=3) as load_pool,
    ):
        wpad = consts.tile([128, 128], BF16)
        wcb = consts.tile([128, 128], BF16)  # cols 64:128 zero (FWL pad)
        wa = consts.tile([128, 128], BF16)   # rows 0:64 + cols 64:128 zero
        bvec = consts.tile([128, 1], F32)   # [b2'(64); b1'(64)]
        bovec = consts.tile([128, 1], F32)  # to (rows 0:64)
        nc.scalar.dma_start(wpad[:], io["wpad"].ap())
        # PE warmup: ~40 short matmuls bring HAM to K=8/8 while E loads
        nc.scalar.dma_start(wcb[:], io["wcb"].ap())
        nc.scalar.dma_start(wa[:], io["wa"].ap())
        nc.scalar.dma_start(bvec[:], io["bvec"].ap())
        nc.scalar.dma_start(bovec[:], io["bovec"].ap())

        # feat_last^T on partitions 64:128; rows 0:64 are zeros (the wa
        # matmul contracts over all 128 partitions, rows 0:64 weights = 0)
        flt = persist.tile([128, NT], BF16)
        # rT: pooled relu'd maxima; rows 0:64 = r2 ch, 64:128 = r1 ch
        rT = persist.tile([128, NT], BF16)
        out_sb = persist.tile([64, NT], BF16)

        e_ap = io["E"].ap()

        with (
            tc.tile_pool(name="ps", bufs=2, space="PSUM") as ps_pool,
            tc.tile_pool(name="zc", bufs=3) as zc_pool,
            tc.tile_pool(name="tr", bufs=2) as tr_pool,
            tc.tile_pool(name="mh", bufs=2) as mh_pool,
        ):
            wps = ps_pool.tile([128, 4, 512], F32, tag="ps")
            for _ in range(28):
                nc.tensor.matmul(wps[:, 0, 0:128], lhsT=wpad[:], rhs=wpad[:],
                                 start=True, stop=True)
            n_loads = (COLS + LOAD_C - 1) // LOAD_C
            for li in range(n_loads):
                c0 = li * LOAD_C
                cw = min(LOAD_C, COLS - c0)
                et = load_pool.tile([128, LOAD_C], BF16, tag="et")
                if li == 0:
                    # per-step DMAs so step 0 can start ASAP; pad-row zeroing
                    # slots in right after step 0's columns
                    nc.sync.dma_start(et[0:ECH, 0:STEP_C], e_ap[:, 0:STEP_C])
                    nc.sync.dma_start(et[ECH:128, :], io["zpad"].ap())
                    for j0 in range(1, LOAD_STEPS):
                        nc.sync.dma_start(
                            et[0:ECH, j0 * STEP_C:(j0 + 1) * STEP_C],
                            e_ap[:, j0 * STEP_C:(j0 + 1) * STEP_C])
                else:
                    if li < 3:
                        nc.sync.dma_start(et[ECH:128, :], io["zpad"].ap())
                    nc.sync.dma_start(et[0:ECH, 0:cw], e_ap[:, c0:c0 + cw])
                if li == 2:
                    nc.sync.dma_start(flt[:], io["fLT"].ap())
                for j in range(cw // STEP_C):
                    i = li * LOAD_STEPS + j
                    ps = ps_pool.tile([128, 4, 512], F32, tag="ps")
                    for m in range(4):
                        nc.tensor.matmul(
                            ps[:, m, :], lhsT=wpad[:],
                            rhs=et[:, j * STEP_C + m * 512:
                                   j * STEP_C + (m + 1) * 512],
                            start=True, stop=True)
                    route = ROUTES[i]
                    rslice = rT[:, i * STEP_T:(i + 1) * STEP_T]
                    if route == "D":
                        psv = ps[:].rearrange("p b (u k) -> p b u k", k=K)
                        mh = mh_pool.tile([128, 4, 32], BF16, tag="mh")
                        nc.vector.tensor_reduce(
                            mh[:], psv[:], axis=mybir.AxisListType.X,
                            op=mybir.AluOpType.max)
                        nc.vector.tensor_scalar(
                            rslice, mh[:].rearrange("p b u -> p (b u)"),
                            bvec[:, 0:1], 0.0,
                            op0=mybir.AluOpType.add, op1=mybir.AluOpType.max)
                    elif i in TREE_PAIR:
                        if TREE_PAIR[i] == 0:
                            zc2 = zc_pool.tile([128, 256, K], BF16, tag="zc2")
                            _emit.zc2 = zc2
                        else:
                            zc2 = _emit.zc2
                        half = TREE_PAIR[i]
                        nc.scalar.activation(
                            zc2[:, half * 128:(half + 1) * 128, :].rearrange(
                                "p g k -> p (g k)"),
                            ps[:].rearrange("p b f -> p (b f)"),
                            mybir.ActivationFunctionType.Relu,
                            bias=bvec[:, 0:1], scale=1.0)
                        if half == 1:
                            r2 = rT[:, (i - 1) * STEP_T:(i + 1) * STEP_T]
                            t8 = tr_pool.tile([128, 256, 8], BF16, tag="p8")
                            nc.vector.tensor_max(
                                t8[:], zc2[:, :, 0:8], zc2[:, :, 8:16])
                            t4 = tr_pool.tile([128, 256, 4], BF16, tag="p4")
                            nc.vector.tensor_max(
                                t4[:], t8[:, :, 0:4], t8[:, :, 4:8])
                            t2 = tr_pool.tile([128, 256, 2], BF16, tag="p2")
                            nc.vector.tensor_max(
                                t2[:], t4[:, :, 0:2], t4[:, :, 2:4])
                            nc.vector.tensor_max(
                                r2, t2[:, :, 0], t2[:, :, 1])
                    else:
                        zc = zc_pool.tile([128, 128, K], BF16, tag="zc")
                        nc.scalar.activation(
                            zc[:].rearrange("p g k -> p (g k)"),
                            ps[:].rearrange("p b f -> p (b f)"),
                            mybir.ActivationFunctionType.Relu,
                            bias=bvec[:, 0:1], scale=1.0)
                        t8 = tr_pool.tile([128, 128, 8], BF16, tag="t8")
                        nc.vector.tensor_max(t8[:], zc[:, :, 0:8], zc[:, :, 8:16])
                        t4 = tr_pool.tile([128, 128, 4], BF16, tag="t4")
                        nc.vector.tensor_max(t4[:], t8[:, :, 0:4], t8[:, :, 4:8])
                        t2 = tr_pool.tile([128, 128, 2], BF16, tag="t2")
                        nc.vector.tensor_max(t2[:], t4[:, :, 0:2], t4[:, :, 2:4])
                        nc.vector.tensor_max(rslice, t2[:, :, 0], t2[:, :, 1])

        # ---- head ----
        with tc.tile_pool(name="hps", bufs=2, space="PSUM") as hps_pool:
            for hi, (c0, w) in enumerate(_head_chunks()):
                hp = hps_pool.tile([128, 512], F32, tag="hp")
                nc.tensor.matmul(hp[:, :w], lhsT=wcb[:],
                                 rhs=rT[:, c0:c0 + w],
                                 start=True, stop=False)
                nc.tensor.matmul(hp[:, :w], lhsT=wa[:],
                                 rhs=flt[:, c0:c0 + w],
                                 start=False, stop=True)
                if hi % 2 == 0:
                    nc.vector.tensor_scalar(
                        out_sb[:, c0:c0 + w], hp[0:64, :w],
                        bovec[0:64, 0:1], 0.0,
                        op0=mybir.AluOpType.add, op1=mybir.AluOpType.max)
                else:
                    nc.scalar.activation(
                        out_sb[:, c0:c0 + w], hp[0:64, :w],
                        mybir.ActivationFunctionType.Relu,
                        bias=bovec[0:64, 0:1], scale=1.0)
                if c0 + w == NT or (c0 + w) % 2048 == 0:
                    d0 = (c0 + w - 1) // 2048 * 2048
                    nc.sync.dma_start(io["oT"].ap()[:, d0:c0 + w],
                                      out_sb[:, d0:c0 + w])


def build():
    nc = bacc.Bacc(
        "TRN2",
        target_bir_lowering=False,
        debug=False,
        enable_asserts=False,
        num_devices=N_CORES,
        num_swdge_queues=4,
    )
    io = {
        "E": nc.dram_tensor("E", [ECH, COLS], BF16, kind="ExternalInput"),
        "zpad": nc.dram_tensor("zpad", [32, LOAD_C], BF16, kind="ExternalInput"),
        "fLT": nc.dram_tensor("fLT", [128, NT], BF16, kind="ExternalInput"),
        "wpad": nc.dram_tensor("wpad", [128, 128], BF16, kind="ExternalInput"),
        "wcb": nc.dram_tensor("wcb", [128, 128], BF16, kind="ExternalInput"),
        "wa": nc.dram_tensor("wa", [128, 128], BF16, kind="ExternalInput"),
        "bvec": nc.dram_tensor("bvec", [128, 1], F32, kind="ExternalInput"),
        "bovec": nc.dram_tensor("bovec", [128, 1], F32, kind="ExternalInput"),
        "oT": nc.dram_tensor("oT", [CG, NT], BF16, kind="ExternalOutput"),
    }
    with tile.TileContext(nc) as tc:
        _emit(tc, io)
    nc.compile()
    return nc


def host_prep_weights(Wg1, bn_g1, Wg2, bn_g2, Wout, bn_out):
    def bn_fold(p):
        g, b, m, v = p[0], p[1], p[2], p[3]
        s = g / np.sqrt(v + EPS)
        return s, b - m * s

    s1, t1 = bn_fold(bn_g1.astype(np.float64))
    s2, t2 = bn_fold(bn_g2.astype(np.float64))
    so, to = bn_fold(bn_out.astype(np.float64))
    sg1 = np.where(s1 >= 0, 1.0, -1.0)
    sg2 = np.where(s2 >= 0, 1.0, -1.0)
    a1, a2 = np.abs(s1), np.abs(s2)

    w1f = Wg1.astype(np.float64) * sg1[None, :]   # [32, 64]
    w2f = Wg2.astype(np.float64) * sg2[None, :]   # [64, 64]

    # W_pad [128, 128]: rows 0:96 = E channels (cols 0:64 = z2 out,
    # 64:128 = z1 out); rows 96:128 zero (rhs pad rows are garbage)
    wpad = np.zeros((128, 128), np.float64)
    wpad[0:C2, 0:CG] = w2f
    wpad[C2:ECH, CG:2 * CG] = w1f

    cl = Wout.shape[0] - 2 * CG
    Wo = Wout.astype(np.float64)
    wB = a1[:, None] * Wo[cl:cl + CG] * so[None, :]
    wC = a2[:, None] * Wo[cl + CG:] * so[None, :]
    # rT rows 0:64 = r2 -> wC; rows 64:128 = r1 -> wB.  Both head weights
    # are padded to [128, 128] (zero cols 64:128; wa also zero rows 0:64)
    # so FWL stays enabled on the head matmuls.
    wcb = np.zeros((128, 128), np.float64)
    wcb[:, 0:CG] = np.concatenate([wC, wB], axis=0)
    wa = np.zeros((128, 128), np.float64)
    wa[CG:128, 0:CG] = Wo[:cl] * so[None, :]

    b2p = (t2 / a2).reshape(CG, 1)
    b1p = (t1 / a1).reshape(CG, 1)
    return dict(
        wpad=wpad.astype(NPBF16),
        wcb=wcb.astype(NPBF16),
        wa=wa.astype(NPBF16),
        bvec=np.concatenate([b2p, b1p], axis=0).astype(np.float32),
        bovec=np.concatenate([to.reshape(CG, 1),
                              np.zeros((CG, 1))]).astype(np.float32),
        zpad=np.zeros((32, LOAD_C), NPBF16),
    )


def _col_maps():
    """E column c -> (target, k). Step i covers targets i*128..i*128+127;
    matmul m covers targets m*32+u (u = 0..31); within-target cols are k."""
    c = np.arange(COLS)
    i = c // STEP_C
    m = (c // 512) % 4
    u = (c // K) % 32
    k = c % K
    t = i * STEP_T + m * 32 + u
    return t.astype(np.int64), k.astype(np.int64)


_T_OF_C, _K_OF_C = _col_maps()


def _host_prep(feat_s1, feat_s2, feat_last, Wg1, bn_g1, Wg2, bn_g2,
               Wout, bn_out, idx_s1, idx_s2):
    common = host_prep_weights(Wg1, bn_g1, Wg2, bn_g2, Wout, bn_out)

    f1b = feat_s1.astype(NPBF16)
    f2b = feat_s2.astype(NPBF16)
    n = feat_last.shape[0]
    n_shard = n // N_CORES

    in_maps = []
    for core in range(N_CORES):
        lo, hi = core * n_shard, (core + 1) * n_shard
        i1 = np.zeros((NT, K), np.int64)
        i1[:n_shard] = idx_s1[lo:hi]
        i2 = np.zeros((NT, K), np.int64)
        i2[:n_shard] = idx_s2[lo:hi]
        E = np.empty((ECH, COLS), NPBF16)
        E[:C2] = f2b[i2[_T_OF_C, _K_OF_C]].T
        E[C2:] = f1b[i1[_T_OF_C, _K_OF_C]].T
        flT = np.zeros((128, NT), NPBF16)
        flT[CG:128, :n_shard] = feat_last[lo:hi].T.astype(NPBF16)
        in_maps.append(dict(common, E=E, fLT=flT))
    return in_maps, n_shard


_BUILD_CACHE = {}


def _ensure_profile_hook():
    """This image's ``antenv`` lacks ``axon_hooks``; concourse's trace=True
    path imports it unconditionally. Provide the module and install the
    ctypes NTFF hook against libaxon_pjrt.so (mirrors trn_boot.py)."""
    import types
    import ctypes
    import contextlib

    try:
        from antenv.axon_hooks import get_axon_ntff_profile_hook  # noqa: F401
        return
    except ImportError:
        pass

    mod = types.ModuleType("antenv.axon_hooks")
    mod._hook = None
    mod.set_axon_ntff_profile_hook = lambda h: setattr(mod, "_hook", h)
    mod.get_axon_ntff_profile_hook = lambda: mod._hook
    sys.modules["antenv.axon_hooks"] = mod
    import antenv
    antenv.axon_hooks = mod

    so_path = "/opt/axon/libaxon_pjrt.so"
    if not os.path.exists(so_path):
        return
    lib = ctypes.CDLL(so_path)
    if not hasattr(lib, "axon_start_nrt_profile"):
        return
    lib.axon_start_nrt_profile.argtypes = [
        ctypes.POINTER(ctypes.c_int64), ctypes.c_size_t,
    ]
    lib.axon_start_nrt_profile.restype = ctypes.c_int64
    lib.axon_stop_nrt_profile.argtypes = [ctypes.c_char_p]
    lib.axon_stop_nrt_profile.restype = ctypes.c_int64

    @contextlib.contextmanager
    def _hook(output_dir, device_ids):
        import jax
        jax.devices()
        if device_ids:
            ids = (ctypes.c_int64 * len(device_ids))(*device_ids)
            rc = lib.axon_start_nrt_profile(ids, len(device_ids))
        else:
            rc = lib.axon_start_nrt_profile(None, 0)
        if rc != 0:
            raise RuntimeError(f"axon_start_nrt_profile rc={rc}")
        try:
            yield
        finally:
            nf = lib.axon_stop_nrt_profile(str(output_dir).encode())
            print(f"profile: {nf} file(s) written to {output_dir}",
                  file=sys.stderr)

    mod.set_axon_ntff_profile_hook(_hook)


def kernel(**inputs):
    from concourse import bass_utils
    from concourse.bass_interp import get_hw_module

    in_maps, n_shard = _host_prep(**inputs)
    if "nc" not in _BUILD_CACHE:
        _BUILD_CACHE["nc"] = build()
    nc = _BUILD_CACHE["nc"]

    old_m = nc.m
    nc.m = get_hw_module(nc.m)
    try:
        trace = os.environ.get("BIGNN_TRACE", "0") == "1"
        if trace:
            _ensure_profile_hook()
        res = bass_utils.run_bass_kernel_spmd(
            nc, in_maps, core_ids=list(range(N_CORES)),
            trace=trace,
            trace_cores=list(range(N_CORES)) if trace else None,
        )
    finally:
        nc.m = old_m

    kernel.last_results = res
    n = inputs["feat_last"].shape[0]
    out = np.empty((n, CG), np.float32)
    for c in range(N_CORES):
        oT = res.results[c]["oT"].astype(np.float32)   # [64, NT]
        out[c * n_shard:(c + 1) * n_shard] = oT.T[:n_shard]
    return out
